# revision 29
# baseline (speedup 1.0000x reference)
"""BrainGCN Trainium2 kernel: 2x GCNConv + 3 FC layers over a 100K-node,
1.6M-edge random graph, distributed over 8 NeuronCores.

Strategy (default mode "v4"; all FLOPs on x-dependent data run on device):
- Nodes (dst) sharded across 8 cores; per core dst nodes packed into 98
  blocks of 128 slots (vector-LPT balanced), SPMD single program.
- Scatter is a PE matmul per 128-edge tile: psum[feat, slot] +=
  matmul(lhsT=rows[128 edges, feat], rhs=S[128 edges, 128 slots]) where S is
  the weighted one-hot "selection" matrix. S tiles are HOST-BUILT (graph
  structure only: slot + norm weight) and streamed sequentially via HWDGE —
  no per-tile DVE one-hot construction (that was 2.5ms of Vector time in
  the old kernel).
- Layer 1 needs raw x[src] rows, so the host also pre-lays the edge-ordered
  row stream R1 (pure row reordering/replication/casting of x, no FLOPs) —
  layer 1 has NO device gather at all, only sequential streaming DMA + PE.
  L1 self-loops are folded into the stream as ordinary edges (w=1/deg).
- Layer 2 gathers h1[src] rows (device-computed) with gpsimd dma_gather
  from bf16 tables (rows padded to 256B). The 98 blocks are split into 4
  chunks; each chunk's h1 rows are AllGathered into a per-chunk Shared
  table as soon as that chunk's L1 epilogues finish, and the gathers for
  chunk j-1 overlap L1 streaming of chunk j (emission interleaved so the
  HWDGE FIFO order matches consumption order). Gather calls stay at 8
  tiles (1024 idxs) — larger calls exceed the SWDGE descriptor-ring
  carveout and hang the Q7 descriptor generator. L2 self-loops use a
  sequential load of the core's own h1 rows + a diagonal S tile.
- FC layers run per-block in feature-major form on the PE.

Host-side work is limited to graph-structure preprocessing (degrees, norms,
permutations, index/metadata/selection arrays, row reordering/casting of x)
and final unpermutation.
"""

import os
import sys
import types

import numpy as np


def _install_ntff_hook():
    """Image's antenv lacks axon_hooks; shim it so trace=True can profile."""
    if "antenv.axon_hooks" in sys.modules:
        return
    mod = types.ModuleType("antenv.axon_hooks")
    mod._hook = None
    mod.set_axon_ntff_profile_hook = lambda h: setattr(mod, "_hook", h)
    mod.get_axon_ntff_profile_hook = lambda: mod._hook
    sys.modules["antenv.axon_hooks"] = mod
    try:
        import antenv
        antenv.axon_hooks = mod
        from trn_agent_boot.trn_boot import _ntff_profile_via_ctypes
        mod.set_axon_ntff_profile_hook(
            _ntff_profile_via_ctypes("/opt/axon/libaxon_pjrt.so")
        )
    except Exception:
        pass


_install_ntff_hook()

import ml_dtypes
import concourse.bacc as bacc
import concourse.bass as bass  # noqa: F401
import concourse.mybir as mybir
import concourse.tile as tile
from concourse.bass_utils import run_bass_kernel_spmd

# ---------------------------------------------------------------- constants
N = 100000
D_IN = 128
H1 = 64
NCORES = 8
SHARD = N // NCORES            # 12500
BLKN = 98                      # blocks of 128 slots per core
SLOTS = BLKN * 128             # 12544
NROWS2 = NCORES * SLOTS        # 100352 rows in the allgathered h1 table
CHUNK = 32                     # tiles per stream DMA (HWDGE)
# SWDGE gather call size: the descriptor carveout holds 256 descs/lane and a
# k-tile gather call needs 8k+1 per lane; 4 queues x 2 dirs split the
# 16KB carveout into 128-desc rings -> 8k+1 <= 128 -> k <= 15.
CHUNK_G = int(os.environ.get("BASS_GCN_CHUNKG", "8"))

# BASS_GCN_MODE:
#   "v4"      — 4 block-chunks; chunked AllGathers interleaved with the
#               bucket gathers on gpsimd so L2 gathers overlap L1's tail.
#   "full"    — 2 signed-idx buckets, single AllGather
#   "posidx"  — 4 positive-idx buckets, single AllGather (baseline-style)
#   "nogather"— debug: skip gather instructions
MODE = os.environ.get("BASS_GCN_MODE", "v4")
if MODE == "full":
    NBUCK2 = 2                 # L2 gather buckets (signed int16 reach 64K)
    B2BASES = [0, 62720]
elif MODE == "v4":
    CH_BLOCKS = [25, 25, 24, 24]   # blocks per chunk
    NBUCK2 = len(CH_BLOCKS)
    _lo = np.cumsum([0] + CH_BLOCKS)
    # chunk j: local slots [_lo[j]*128, _lo[j+1]*128); global table rows
    # [row_base[j], row_base[j] + 8*len_j*128) with per-core stripes
    CH_SLOT_LO = [int(v) * 128 for v in _lo[:-1]]
    CH_SLOT_HI = [int(v) * 128 for v in _lo[1:]]
    CH_ROW_BASE = [0] * NBUCK2
    for _j in range(1, NBUCK2):
        CH_ROW_BASE[_j] = CH_ROW_BASE[_j - 1] + NCORES * (
            CH_SLOT_HI[_j - 1] - CH_SLOT_LO[_j - 1])
    CHUNK_OF_BLOCK = np.repeat(np.arange(NBUCK2), CH_BLOCKS)
    B2BASES = CH_ROW_BASE
else:
    NBUCK2 = 4
    B2BASES = [0, 30000, 60000, 90000]

NQUEUES = int(os.environ.get("BASS_GCN_QUEUES", "4"))
BF16 = ml_dtypes.bfloat16

LAST_EXEC_TIME_NS = None       # filled when BASS_GCN_TRACE=1
LAST_RESULTS = None


# ------------------------------------------------------------- host planning
def _lpt_assign_vec(loads):
    """Pack nodes into BLKN blocks x 128 slots, balancing the per-column load
    vectors (sum-of-squares greedy, descending total load)."""
    n = loads.shape[0]
    order = np.argsort(-loads.sum(1), kind="stable")
    block_loads = np.zeros((BLKN, loads.shape[1]), np.float64)
    used = np.zeros(BLKN, np.int64)
    pos = np.empty(n, np.int64)
    for i in order:
        li = loads[i]
        cand = block_loads + li
        score = np.einsum("ij,ij->i", cand, cand)
        score[used >= 128] = np.inf
        b = int(np.argmin(score))
        pos[i] = b * 128 + used[b]
        block_loads[b] += li
        used[b] += 1
    return pos


def _bucket2_of(rows):
    out = np.zeros(len(rows), np.int64)
    for i, b in enumerate(B2BASES[1:], 1):
        out += rows >= b
    return out


def _pack_tiles(arr_PT, P):
    """[P_total, width] edge-major -> [128, P_total] tile-interleaved lhsT
    layout: column t*width.. holds tile t, partition e = edge within tile."""
    ntiles = P // 128
    w = arr_PT.shape[1]
    return np.ascontiguousarray(
        arr_PT.reshape(ntiles, 128, w).transpose(1, 0, 2).reshape(128, ntiles * w)
    )


def _plan(src, dst, x):
    """Full host-side graph preprocessing."""
    deg = (np.bincount(dst, minlength=N) + 1.0).astype(np.float64)
    dinv = 1.0 / np.sqrt(deg)
    w_edge = (dinv[src] * dinv[dst]).astype(np.float32)
    w_self = (1.0 / deg).astype(np.float32)

    core_of = dst // SHARD

    # per-node load columns for LPT balancing:
    # col 0: L1 edges (in-degree + self), cols 1..2: L2 edges by src bucket
    indeg = np.bincount(dst, minlength=N).astype(np.float64)
    loads1 = (indeg + 1.0)[:, None]
    if MODE == "v4":
        # bucket of an edge = chunk of the src's BLOCK, which is decided by
        # the LPT below — unknowable here, and uniform per chunk anyway.
        loads = loads1
    else:
        # L2 bucket of an edge depends on where src's core range maps;
        # approximate by spreading the src core's row window over buckets.
        bounds = list(B2BASES) + [NROWS2]
        frac = np.zeros((NCORES, NBUCK2), np.float64)
        for c in range(NCORES):
            lo, hi = c * SLOTS, (c + 1) * SLOTS
            for be in range(NBUCK2):
                frac[c, be] = max(
                    0, min(hi, bounds[be + 1]) - max(lo, bounds[be])) / SLOTS
        loads2 = np.zeros((N, NBUCK2), np.float64)
        src_core = src // SHARD
        for be in range(NBUCK2):
            np.add.at(loads2, (dst, be), frac[src_core, be])
        loads = np.concatenate([loads1, loads2], axis=1)

    pos_local = np.empty(N, np.int64)
    node_of_pos = np.full((NCORES, SLOTS), -1, np.int64)
    for c in range(NCORES):
        nodes = np.arange(c * SHARD, (c + 1) * SHARD)
        p = _lpt_assign_vec(loads[nodes])
        pos_local[nodes] = p
        node_of_pos[c, p] = nodes
    if MODE == "v4":
        # chunk-major global table layout: chunk j holds all 8 cores'
        # stripes for its block range, so AG_j can complete it on its own
        nodes_core = np.arange(N) // SHARD
        blk_of_node = pos_local // 128
        ch_of_node = CHUNK_OF_BLOCK[blk_of_node]
        ch_len = np.array(
            [CH_SLOT_HI[j] - CH_SLOT_LO[j] for j in range(NBUCK2)], np.int64)
        pos_global = (np.asarray(CH_ROW_BASE, np.int64)[ch_of_node]
                      + nodes_core * ch_len[ch_of_node]
                      + pos_local - np.asarray(CH_SLOT_LO, np.int64)[ch_of_node])
    else:
        pos_global = (np.arange(N) // SHARD) * SLOTS + pos_local

    # ---- L1 edge list: original edges + self loops
    allN = np.arange(N, dtype=np.int64)
    src1 = np.concatenate([src, allN])
    dst1 = np.concatenate([dst, allN])
    w1 = np.concatenate([w_edge, w_self.astype(np.float32)])
    core1 = dst1 // SHARD
    blk1 = pos_local[dst1] // 128

    # ---- L2 rows (positions in the padded global table)
    rows2 = pos_global[src]
    be2 = _bucket2_of(rows2)
    blk2 = pos_local[dst] // 128

    # per-(core, group) counts -> shared tile counts T1 / T2
    cnt1 = np.zeros((NCORES, BLKN), np.int64)
    cnt2 = np.zeros((NCORES, NBUCK2, BLKN), np.int64)
    for c in range(NCORES):
        m1 = core1 == c
        cnt1[c] = np.bincount(blk1[m1], minlength=BLKN)
        m2 = core_of == c
        cnt2[c] = np.bincount(
            be2[m2] * BLKN + blk2[m2], minlength=NBUCK2 * BLKN
        ).reshape(NBUCK2, BLKN)
    T1 = np.ceil(cnt1.max(axis=0) / 128).astype(np.int64)          # [BLKN]
    T2 = np.ceil(cnt2.max(axis=0) / 128).astype(np.int64)          # [NBUCK2, BLKN]
    P1 = 128 * int(T1.sum())
    P2 = 128 * int(T2.sum())

    # ---- L2 tile schedule + gather call breaks (shared across cores).
    sched2 = []
    groups = []  # (be, b, ntiles) in schedule order, nonzero only
    for be in range(NBUCK2):
        for b in range(BLKN):
            tt = int(T2[be][b])
            if tt > 0:
                groups.append((be, b, tt))
            for t in range(tt):
                sched2.append((be, b, t))
    calls2 = []  # [gi, k, bucket]
    if MODE == "full":
        # Signed idxs: calls break ONLY at group boundaries so each call ends
        # on its final group's max row (or its pads) — guarantees the call's
        # trailing index is non-negative (Q7 strips trailing negatives).
        assert int(T2.max()) <= CHUNK_G
        gi = 0
        g = 0
        while g < len(groups):
            be0 = groups[g][0]
            k = 0
            while (g < len(groups) and groups[g][0] == be0
                   and k + groups[g][2] <= CHUNK_G):
                k += groups[g][2]
                g += 1
            calls2.append([gi, k, be0])
            gi += k
    else:
        # Positive idxs never strip: break calls anywhere within a bucket.
        gi = 0
        while gi < len(sched2):
            be0 = sched2[gi][0]
            k = 1
            while (k < CHUNK_G and gi + k < len(sched2)
                   and sched2[gi + k][0] == be0):
                k += 1
            calls2.append([gi, k, be0])
            gi += k

    # ---- per-core streams
    x_bf = np.asarray(x, np.float32).astype(BF16)
    dest_base1 = np.zeros(BLKN + 1, np.int64)
    np.cumsum(128 * T1, out=dest_base1[1:])
    Tflat2 = T2.ravel()
    dest_base2 = np.zeros(NBUCK2 * BLKN + 1, np.int64)
    np.cumsum(128 * Tflat2, out=dest_base2[1:])

    # per-call max row across all cores (for the shared per-call base)
    call_gmax = np.zeros(len(calls2), np.int64)

    percore = []
    for c in range(NCORES):
        # L1 stream
        m = core1 == c
        key = blk1[m]
        order = np.argsort(key, kind="stable")
        srows = src1[m][order]
        sslot = (pos_local[dst1[m]] % 128)[order]
        sw = w1[m][order]
        skey = key[order]
        counts = np.bincount(skey, minlength=BLKN)
        starts = np.zeros(BLKN + 1, np.int64)
        np.cumsum(counts, out=starts[1:])
        rank = np.arange(len(order)) - starts[skey]
        dest = dest_base1[skey] + rank
        rows_p = np.zeros(P1, np.int64)
        slot_p = np.zeros(P1, np.int64)
        w_p = np.zeros(P1, np.float32)
        rows_p[dest] = srows
        slot_p[dest] = sslot
        w_p[dest] = sw
        R1 = _pack_tiles(x_bf[rows_p], P1)                          # [128, P1]
        S1e = np.zeros((P1, 128), np.float32)
        S1e[np.arange(P1), slot_p] = w_p
        S1 = _pack_tiles(S1e.astype(BF16), P1)                      # [128, P1]
        del S1e

        # L2 stream
        m2 = core_of == c
        r2 = rows2[m2]
        key2 = be2[m2] * BLKN + blk2[m2]
        order2 = np.lexsort((r2, key2))
        sr2 = r2[order2]
        sslot2 = (pos_local[dst[m2]] % 128)[order2]
        sw2 = w_edge[m2][order2]
        skey2 = key2[order2]
        counts2l = np.bincount(skey2, minlength=NBUCK2 * BLKN)
        starts2 = np.zeros(NBUCK2 * BLKN + 1, np.int64)
        np.cumsum(counts2l, out=starts2[1:])
        rank2 = np.arange(len(order2)) - starts2[skey2]
        dest2 = dest_base2[skey2] + rank2
        rows2_p = np.full(P2, -1, np.int64)                         # -1 = pad
        slot2_p = np.zeros(P2, np.int64)
        w2_p = np.zeros(P2, np.float32)
        rows2_p[dest2] = sr2
        slot2_p[dest2] = sslot2
        w2_p[dest2] = sw2
        S2e = np.zeros((P2, 128), np.float32)
        S2e[np.arange(P2), slot2_p] = w2_p
        S2 = _pack_tiles(S2e.astype(BF16), P2)                      # [128, P2]
        del S2e

        percore.append({"R1": R1, "S1": S1, "rows2_p": rows2_p, "S2": S2})

        for ci, (gi0, k, _be) in enumerate(calls2):
            seg = rows2_p[gi0 * 128: (gi0 + k) * 128]
            real = seg[seg >= 0]
            if len(real):
                call_gmax[ci] = max(call_gmax[ci], int(real.max()))

    # per-call shared bases, then per-core int16 idx streams
    if MODE == "full":
        call_base = np.maximum(0, call_gmax - 32767)
    else:
        call_base = np.array([B2BASES[be] for (_g, _k, be) in calls2], np.int64)
    for ci in range(len(calls2)):
        calls2[ci].append(int(call_base[ci]))

    for c in range(NCORES):
        rows2_p = percore[c].pop("rows2_p")
        idx = np.zeros(P2, np.int64)
        for (gi0, k, _be, base) in calls2:
            lo, hi = gi0 * 128, (gi0 + k) * 128
            seg = rows2_p[lo:hi]
            real = seg[seg >= 0]
            cmax = int(real.max()) if len(real) else base
            pad_idx = min(max(cmax - base, 0), 32767)
            vals = np.where(seg >= 0, seg - base, pad_idx)
            assert vals.min() >= -32768 and vals.max() <= 32767, (
                c, gi0, vals.min(), vals.max())
            assert vals[-1] >= 0, (c, gi0)   # trailing-negative strip guard
            if MODE != "full":
                assert vals.min() >= 0, (c, gi0, vals.min())
            idx[lo:hi] = vals
        idx16 = idx.astype(np.int16)
        idx_wrapped = np.ascontiguousarray(
            np.tile(idx16.reshape(-1, 16).T, (8, 1)))               # [128, P2//16]
        percore[c]["idx2"] = idx_wrapped

        # L2 self-loop diagonal S tiles
        wcol = np.zeros(SLOTS, np.float32)
        valid = node_of_pos[c] >= 0
        wcol[valid] = w_self[node_of_pos[c][valid]]
        Sself = np.zeros((SLOTS, 128), np.float32)
        Sself[np.arange(SLOTS), np.arange(SLOTS) % 128] = wcol
        percore[c]["S2self"] = _pack_tiles(Sself.astype(BF16), SLOTS)

    return percore, T1, T2, sched2, calls2, node_of_pos


# ------------------------------------------------------------ device program
def _build_program(T1, T2, sched2, calls2, wshapes):
    f32 = mybir.dt.float32
    bf = mybir.dt.bfloat16
    nc = bacc.Bacc("TRN2", num_swdge_queues=NQUEUES)

    P1 = 128 * int(T1.sum())
    P2 = 128 * int(T2.sum())

    R1_d = nc.dram_tensor("R1", [128, P1], bf, kind="ExternalInput")
    S1_d = nc.dram_tensor("S1", [128, P1], bf, kind="ExternalInput")
    idx2_d = nc.dram_tensor("idx2", [128, P2 // 16], mybir.dt.int16,
                            kind="ExternalInput")
    S2_d = nc.dram_tensor("S2", [128, P2], bf, kind="ExternalInput")
    S2self_d = nc.dram_tensor("S2self", [128, SLOTS], bf, kind="ExternalInput")
    ident_d = nc.dram_tensor("ident", [128, 128], f32, kind="ExternalInput")
    wdr = {}
    for name, shp in wshapes.items():
        wdr[name] = nc.dram_tensor(name, list(shp), f32, kind="ExternalInput")
    y_d = nc.dram_tensor("y", [BLKN, 128], f32, kind="ExternalOutput")

    # L1 tile schedule
    sched1 = []
    for b in range(BLKN):
        for t in range(int(T1[b])):
            sched1.append((b, t))

    # last nonzero L2 group per block (self tile + epilogue hook)
    last_be = np.full(BLKN, -1, np.int64)
    ngroups = np.zeros(BLKN, np.int64)
    for b in range(BLKN):
        nz = [be for be in range(NBUCK2) if T2[be][b] > 0]
        ngroups[b] = len(nz)
        if nz:
            last_be[b] = nz[-1]

    with tile.TileContext(nc) as tc:
        with (
            tc.tile_pool(name="cst", bufs=1) as cst,
            tc.tile_pool(name="st1", bufs=2) as st1,
            tc.tile_pool(name="gp", bufs=(6 if CHUNK_G <= 12 else 3)) as gp,
            tc.tile_pool(name="s2p", bufs=(6 if CHUNK_G <= 12 else 3)) as s2p,
            tc.tile_pool(name="sfp", bufs=2) as sfp,
            tc.tile_pool(name="accp", bufs=1) as accp,
            tc.tile_pool(name="hp", bufs=4) as hp,
            tc.tile_pool(name="ps_run", bufs=5, space="PSUM") as ps_run,
            tc.tile_pool(name="ps_epi", bufs=3, space="PSUM") as ps_epi,
            tc.tile_pool(name="dram", bufs=1, space="DRAM") as dram,
        ):
            ident_t = cst.tile([128, 128], f32)
            nc.sync.dma_start(ident_t[:], ident_d[:])
            wt = {}
            for name in wshapes:
                wt[name] = cst.tile(list(wshapes[name]), f32, name=f"w_{name}")
                nc.sync.dma_start(wt[name][:], wdr[name][:])

            h1_shard = dram.tile([SLOTS, 128], bf)
            if MODE == "v4":
                # Shared DRAM allows a single writer inst: one tile per AG
                h1_fulls = [
                    dram.tile(
                        [NCORES * (CH_SLOT_HI[j] - CH_SLOT_LO[j]), 128], bf,
                        addr_space="Shared", name=f"h1_full{j}")
                    for j in range(NBUCK2)
                ]
                h1_full = None
            else:
                h1_full = dram.tile([NROWS2, 128], bf, addr_space="Shared")

            def epi1(b, psum):
                agg = hp.tile([128, 128], f32, tag="agg")
                nc.vector.tensor_copy(agg[:], psum[:])
                eps = ps_epi.tile([H1, 128], f32, tag="eps")
                nc.tensor.matmul(eps[:], wt["cW0"][:], agg[:],
                                 start=True, stop=True)
                h1T = hp.tile([H1, 128], f32, tag="h1T")
                nc.scalar.activation(
                    h1T[:], eps[:], mybir.ActivationFunctionType.Tanh,
                    bias=wt["cb0"][:, 0:1],
                )
                tp = ps_epi.tile([128, H1], f32, tag="eps")
                nc.tensor.transpose(tp[:], h1T[:], ident_t[:H1, :H1])
                h1n = hp.tile([128, H1], bf, tag="h1n")
                nc.vector.tensor_copy(h1n[:], tp[:])
                nc.scalar.dma_start(
                    h1_shard[b * 128: (b + 1) * 128, 0:H1], h1n[:]
                )

            # ---------------- layer 1: pure streaming, no gather ----------
            n1 = len(sched1)

            def emit_l1_range(t_lo, t_hi):
                gi = t_lo
                cur_ps = None
                while gi < t_hi:
                    k = min(CHUNK, t_hi - gi)
                    r1c = st1.tile([128, CHUNK * 128], bf, tag="r1c")
                    nc.sync.dma_start(r1c[:, : k * 128],
                                      R1_d[:, gi * 128: (gi + k) * 128])
                    s1c = st1.tile([128, CHUNK * 128], bf, tag="s1c")
                    nc.scalar.dma_start(s1c[:, : k * 128],
                                        S1_d[:, gi * 128: (gi + k) * 128])
                    for tl in range(k):
                        b, t = sched1[gi + tl]
                        if t == 0:
                            cur_ps = ps_run.tile([128, 128], f32, tag="runps")
                        nc.tensor.matmul(
                            cur_ps[:],
                            r1c[:, tl * 128: (tl + 1) * 128],
                            s1c[:, tl * 128: (tl + 1) * 128],
                            start=(t == 0), stop=(t == int(T1[b]) - 1),
                        )
                        if t == int(T1[b]) - 1:
                            epi1(b, cur_ps)
                    gi += k

            # ---------------- exchange h1 ---------------------------------
            def emit_ag(j):
                """AllGather chunk j of h1 (whole table when j is None)."""
                if j is None:
                    ins_ap, outs_ap = h1_shard.opt(), h1_full.opt()
                else:
                    lo, hi = CH_SLOT_LO[j], CH_SLOT_HI[j]
                    ins_ap = h1_shard[lo:hi, :]
                    outs_ap = h1_fulls[j].opt()
                nc.gpsimd.collective_compute(
                    "AllGather",
                    mybir.AluOpType.bypass,
                    ins=[ins_ap],
                    outs=[outs_ap],
                    replica_groups=[list(range(NCORES))],
                )

            # ---------------- layer 2: gather + scatter -------------------
            acc_tiles = {}
            groups_done = np.zeros(BLKN, np.int64)

            def epi2(b, rhs_sb):
                e1 = ps_epi.tile([H1, 128], f32, tag="eps")
                nc.tensor.matmul(e1[:], wt["cW1"][:], rhs_sb[:],
                                 start=True, stop=True)
                h2T = hp.tile([H1, 128], f32, tag="h2T")
                nc.scalar.activation(
                    h2T[:], e1[:], mybir.ActivationFunctionType.Tanh,
                    bias=wt["cb1"][:, 0:1],
                )
                e2 = ps_epi.tile([H1, 128], f32, tag="eps")
                nc.tensor.matmul(e2[:], wt["fW0"][:], h2T[:],
                                 start=True, stop=True)
                h3T = hp.tile([H1, 128], f32, tag="h3T")
                nc.scalar.activation(
                    h3T[:], e2[:], mybir.ActivationFunctionType.Tanh,
                    bias=wt["fb0"][:, 0:1],
                )
                e3 = ps_epi.tile([32, 128], f32, tag="eps")
                nc.tensor.matmul(e3[:], wt["fW1"][:], h3T[:],
                                 start=True, stop=True)
                h4T = hp.tile([32, 128], f32, tag="h4T")
                nc.scalar.activation(
                    h4T[:], e3[:], mybir.ActivationFunctionType.Tanh,
                    bias=wt["fb1"][:, 0:1],
                )
                e4 = ps_epi.tile([1, 128], f32, tag="eps")
                nc.tensor.matmul(e4[:], wt["fW2"][:], h4T[:],
                                 start=True, stop=True)
                yrow = hp.tile([1, 128], f32, tag="yrow")
                nc.vector.tensor_scalar_add(yrow[:], e4[:], wt["fb2"][0:1, 0:1])
                nc.sync.dma_start(y_d[b: b + 1, :], yrow[:])

            def finish_group(b, be, psum):
                """Close the (be, b) psum run: self tile if last group, then
                accumulate / launch the epilogue."""
                is_last = be == last_be[b]
                if is_last:
                    selfr = sfp.tile([128, 128], bf, tag="selfr")
                    nc.scalar.dma_start(
                        selfr[:], h1_shard[b * 128: (b + 1) * 128, :])
                    s2s = sfp.tile([128, 128], bf, tag="s2s")
                    nc.sync.dma_start(
                        s2s[:], S2self_d[:, b * 128: (b + 1) * 128])
                    nc.tensor.matmul(psum[:], selfr[:, 0:H1], s2s[:],
                                     start=False, stop=True)
                    if b in acc_tiles:
                        nc.vector.tensor_add(
                            acc_tiles[b][:], acc_tiles[b][:], psum[:])
                        epi2(b, acc_tiles[b])
                    else:
                        agg2 = hp.tile([H1, 128], f32, tag="agg2")
                        nc.vector.tensor_copy(agg2[:], psum[:])
                        epi2(b, agg2)
                else:
                    if b not in acc_tiles:
                        acc_tiles[b] = accp.tile(
                            [H1, 128], f32, tag=f"acc{b}", name=f"acc_{b}")
                        nc.vector.tensor_copy(acc_tiles[b][:], psum[:])
                    else:
                        nc.vector.tensor_add(
                            acc_tiles[b][:], acc_tiles[b][:], psum[:])

            qctr = [0]

            def emit_l2_calls(call_list):
                cur_ps2 = None
                for (gi0, k, be0, base) in call_list:
                    idx_t = gp.tile([128, CHUNK_G * 8], mybir.dt.int16,
                                    tag="idx2")
                    nc.scalar.dma_start(idx_t[:, : k * 8],
                                        idx2_d[:, gi0 * 8: (gi0 + k) * 8])
                    s2c = s2p.tile([128, CHUNK_G * 128], bf, tag="s2c")
                    nc.sync.dma_start(s2c[:, : k * 128],
                                      S2_d[:, gi0 * 128: (gi0 + k) * 128])
                    gat = gp.tile([128, CHUNK_G, 128], bf, tag="gat")
                    if MODE == "v4":
                        table_ap = h1_fulls[be0][:, :]
                    else:
                        table_ap = h1_full[base:NROWS2, :]
                    if MODE != "nogather":
                        nc.gpsimd.dma_gather(
                            gat[:, :k, :], table_ap,
                            idx_t[:, : k * 8], k * 128, k * 128, 128,
                            queue_num=qctr[0] % NQUEUES,
                        )
                    else:
                        nc.gpsimd.memset(gat[:, :k, :], 0.0)
                    qctr[0] += 1
                    for tl in range(k):
                        be, b, t = sched2[gi0 + tl]
                        if t == 0:
                            cur_ps2 = ps_run.tile([H1, 128], f32, tag="runps")
                        group_end = t == int(T2[be][b]) - 1
                        stop_here = group_end and (be != last_be[b])
                        nc.tensor.matmul(
                            cur_ps2[:], gat[:, tl, 0:H1],
                            s2c[:, tl * 128: (tl + 1) * 128],
                            start=(t == 0), stop=stop_here,
                        )
                        if group_end:
                            finish_group(b, be, cur_ps2)

            if MODE == "v4" and os.environ.get("BASS_GCN_PIPE", "1") == "1":
                # Pipelined emission: L1 chunk j streams while bucket j-1's
                # gathers run; AG_j follows its chunk's epilogue writes.
                T1cum = np.zeros(BLKN + 1, np.int64)
                np.cumsum(T1, out=T1cum[1:])
                emit_l1_range(0, int(T1cum[CH_SLOT_HI[0] // 128]))
                emit_ag(0)
                for j in range(1, NBUCK2):
                    blo, bhi = CH_SLOT_LO[j] // 128, CH_SLOT_HI[j] // 128
                    emit_l1_range(int(T1cum[blo]), int(T1cum[bhi]))
                    emit_l2_calls([c for c in calls2 if c[2] == j - 1])
                    emit_ag(j)
                emit_l2_calls([c for c in calls2 if c[2] == NBUCK2 - 1])
            elif MODE == "v4":
                # serial emission: all of L1, then per-bucket AG + gathers
                emit_l1_range(0, n1)
                for j in range(NBUCK2):
                    emit_ag(j)
                    emit_l2_calls([c for c in calls2 if c[2] == j])
            else:
                emit_l1_range(0, n1)
                emit_ag(None)
                emit_l2_calls(calls2)

            # blocks with no gathered L2 edges at all: self-only run
            for b in range(BLKN):
                if ngroups[b] == 0:
                    selfr = sfp.tile([128, 128], bf, tag="selfr")
                    nc.scalar.dma_start(
                        selfr[:], h1_shard[b * 128: (b + 1) * 128, :])
                    s2s = sfp.tile([128, 128], bf, tag="s2s")
                    nc.sync.dma_start(
                        s2s[:], S2self_d[:, b * 128: (b + 1) * 128])
                    ps = ps_run.tile([H1, 128], f32, tag="runps")
                    nc.tensor.matmul(ps[:], selfr[:, 0:H1], s2s[:],
                                     start=True, stop=True)
                    agg2 = hp.tile([H1, 128], f32, tag="agg2")
                    nc.vector.tensor_copy(agg2[:], ps[:])
                    epi2(b, agg2)

    nc.compile()
    return nc


# ------------------------------------------------------------------- driver
def kernel(**inputs):
    global LAST_EXEC_TIME_NS, LAST_RESULTS

    x = np.ascontiguousarray(np.asarray(inputs["x"], np.float32))
    ei = np.asarray(inputs["edge_index"], np.int64)
    src, dst = ei[0], ei[1]

    weights = {
        "cW0": np.ascontiguousarray(np.asarray(inputs["cW0"], np.float32)),
        "cb0": np.asarray(inputs["cb0"], np.float32).reshape(H1, 1),
        "cW1": np.ascontiguousarray(np.asarray(inputs["cW1"], np.float32)),
        "cb1": np.asarray(inputs["cb1"], np.float32).reshape(H1, 1),
        "fW0": np.ascontiguousarray(np.asarray(inputs["fW0"], np.float32)),
        "fb0": np.asarray(inputs["fb0"], np.float32).reshape(H1, 1),
        "fW1": np.ascontiguousarray(np.asarray(inputs["fW1"], np.float32)),
        "fb1": np.asarray(inputs["fb1"], np.float32).reshape(32, 1),
        "fW2": np.ascontiguousarray(np.asarray(inputs["fW2"], np.float32)),
        "fb2": np.asarray(inputs["fb2"], np.float32).reshape(1, 1),
    }

    percore, T1, T2, sched2, calls2, node_of_pos = _plan(src, dst, x)

    nc = _build_program(T1, T2, sched2, calls2,
                        {k: v.shape for k, v in weights.items()})

    ident = np.eye(128, dtype=np.float32)
    in_maps = []
    for c in range(NCORES):
        m = {"R1": percore[c]["R1"], "S1": percore[c]["S1"],
             "idx2": percore[c]["idx2"], "S2": percore[c]["S2"],
             "S2self": percore[c]["S2self"], "ident": ident}
        m.update(weights)
        in_maps.append(m)

    trace = os.environ.get("BASS_GCN_TRACE") == "1"
    res = run_bass_kernel_spmd(nc, in_maps, list(range(NCORES)), trace=trace)
    if trace:
        LAST_EXEC_TIME_NS = res.exec_time_ns
    LAST_RESULTS = res

    out = np.zeros((N, 1), np.float32)
    for c in range(NCORES):
        yflat = res.results[c]["y"].reshape(SLOTS)
        valid = node_of_pos[c] >= 0
        out[node_of_pos[c][valid], 0] = yflat[valid]
    return out
